# revision 9
# baseline (speedup 1.0000x reference)
"""Causal self-attention (Q=K=V=x, unscaled) on 8 trn2 NeuronCores.

x: [8, 2048, 512] f32. Data-parallel over batch: core b computes batch
element b entirely on-chip. fp16 operands for both matmuls (f32 PSUM
accumulation, f32 softmax arithmetic):

  load    x -> fp16 DRAM staging via casting SWDGE DMA (4 chunks);
          xh  [128,16,512] = x in fp16, t-blocked  (one DMA per chunk)
          xth [128,4,2048] = x.T in fp16 via xbar DMA-transpose
          (one blocked-transpose DMA per chunk; PE/DVE untouched)
  scores  S = x @ x.T causal lower triangle in 512-col PSUM chunks
  softmax causal mask added in-PSUM (DVE), per-chunk partial row-maxes
          (DVE), exp reads PSUM directly (ACT) -> fp16 P strip with
          fused per-chunk row-sums
  out     whole P strip transposed in one blocked xbar DMA, P @ x
          accumulates in PSUM, 1/rowsum fused into the output copy
"""

import numpy as np

import concourse.bass as bass
import concourse.mybir as mybir
import concourse.tile as tile
from concourse import bacc
from concourse.bass_utils import run_bass_kernel_spmd
from concourse.masks import make_causal_mask

B, S, D = 8, 2048, 512
P = 128
NQ = S // P  # 16 q-blocks of 128 rows
ND = D // P  # 4 contraction chunks of 128
CW = 512  # score chunk width (one PSUM bank of f32)
F32 = mybir.dt.float32
F16 = mybir.dt.float16
MASK_VAL = -1e30


def _emit(nc: bass.Bass, reps: int = 1):
    x_d = nc.dram_tensor("x", [S, D], F32, kind="ExternalInput").ap()
    o_d = nc.dram_tensor("out", [S, D], F32, kind="ExternalOutput").ap()

    with tile.TileContext(nc) as tc:
        with (
            tc.tile_pool(name="const", bufs=1) as cpool,
            tc.tile_pool(name="dram", bufs=1, space="DRAM") as dpool,
            tc.tile_pool(name="xsb", bufs=1) as x_pool,
            tc.tile_pool(name="pstrip", bufs=2) as sc_pool,
            tc.tile_pool(name="pts", bufs=2) as pt_pool,
            tc.tile_pool(name="ob", bufs=2) as o_pool,
            tc.tile_pool(name="stat", bufs=2) as st_pool,
            tc.tile_pool(name="ps_sc", bufs=5, space="PSUM") as ps_sc,
            tc.tile_pool(name="ps_pv", bufs=2, space="PSUM") as ps_pv,
        ):
            cmask = cpool.tile([P, P], F32, tag="cmask")
            make_causal_mask(nc, cmask[:], mask_val=MASK_VAL)

            if reps > 1:
                # benchmarking only: repeat the whole body in a HW loop
                import contextlib  # noqa: F401

                loop_cm = tc.For_i(
                    0, reps, 1, hint_engines=(mybir.EngineType.PE,)
                )
            else:
                import contextlib

                loop_cm = contextlib.nullcontext()
            with loop_cm:
                _emit_body(nc, tc, x_d, o_d, cmask, dpool, x_pool, sc_pool,
                           pt_pool, o_pool, st_pool, ps_sc, ps_pv)


def _emit_body(nc, tc, x_d, o_d, cmask, dpool, x_pool, sc_pool, pt_pool,
               o_pool, st_pool, ps_sc, ps_pv):
    xf16 = dpool.tile([S, D], F16, tag="xf16")
    xh = x_pool.tile([P, NQ, D], F16, tag="xh")
    xth = x_pool.tile([P, ND, S], F16, tag="xth")
    x_blk = x_d.rearrange("(n p) d -> p n d", p=P)
    xf_blk = xf16.rearrange("(n p) d -> p n d", p=P)

    def emit_setup_group(tg):
        # rows tg*512 .. (tg+1)*512: cast to fp16 DRAM staging, then load
        # natural-layout block and blocked-transpose block, all DMA-only.
        r0, r1 = tg * 4 * P, (tg + 1) * 4 * P
        nc.gpsimd.dma_start(xf16[r0:r1, :], x_d[r0:r1, :])
        nc.sync.dma_start(
            xh[:, tg * 4 : (tg + 1) * 4, :], xf_blk[:, tg * 4 : (tg + 1) * 4, :]
        )
        nc.sync.dma_start_transpose(
            xth[:, :, r0:r1], xf16[r0:r1, :]
        )

    # Software pipeline: stage s emits scores+softmax for q-block s and
    # PV for q-block s-1, so DVE/ACT softmax of one block overlaps PE
    # matmuls of the next.
    state = [None] * NQ
    for step in range(NQ + 1):
        if step < NQ:
            qi = step
            if qi % 4 == 0:
                emit_setup_group(qi // 4)
            width = (qi + 1) * P
            nfull, rem = divmod(width, CW)
            widths = [CW] * nfull + ([rem] if rem else [])
            nch = len(widths)
            pstrip = sc_pool.tile([P, S], F16, tag="pstrip")
            pmax = st_pool.tile([P, ND], F32, tag="pmax")
            chunks = []
            for c, cw in enumerate(widths):
                ps = ps_sc.tile([P, CW], F32, tag="ps", name=f"ps{qi}_{c}")
                for dk in range(ND):
                    nc.tensor.matmul(
                        ps[:, :cw],
                        xth[:, dk, qi * P : (qi + 1) * P],
                        xth[:, dk, c * CW : c * CW + cw],
                        start=(dk == 0),
                        stop=(dk == ND - 1),
                    )
                lo = c * CW
                if lo + cw > qi * P:
                    # chunk holds the diagonal 128x128 tile: apply the
                    # causal mask in place in PSUM
                    doff = qi * P - lo
                    nc.vector.tensor_add(
                        ps[:, doff : doff + P],
                        ps[:, doff : doff + P],
                        cmask[:],
                    )
                nc.vector.reduce_max(
                    pmax[:, c : c + 1],
                    ps[:, :cw],
                    axis=mybir.AxisListType.X,
                )
                chunks.append((ps, lo, cw))
            nmax = st_pool.tile([P, 1], F32, tag="nmax")
            nc.vector.reduce_max(
                nmax[:],
                pmax[:, :nch],
                axis=mybir.AxisListType.X,
                negate=True,
            )
            psums = st_pool.tile([P, ND], F32, tag="psums")
            for c, (ps, lo, cw) in enumerate(chunks):
                nc.scalar.activation(
                    pstrip[:, lo : lo + cw],
                    ps[:, :cw],
                    mybir.ActivationFunctionType.Exp,
                    bias=nmax[:],
                    scale=1.0,
                    accum_out=psums[:, c : c + 1],
                )
            rsum = st_pool.tile([P, 1], F32, tag="rsum")
            nc.vector.reduce_sum(
                rsum[:], psums[:, :nch], axis=mybir.AxisListType.X
            )
            rcp = st_pool.tile([P, 1], F32, tag="rcp")
            nc.vector.reciprocal(rcp[:], rsum[:])
            state[qi] = (pstrip, rcp)

        if step >= 1:
            qi2 = step - 1
            pstrip, rcp = state[qi2]
            state[qi2] = None
            ntile = qi2 + 1
            # whole-strip blocked transpose: pts[:, j, :] = P-tile j ^T
            pts = pt_pool.tile([P, NQ, P], F16, tag="pts")
            nc.sync.dma_start_transpose(
                pts[:, :ntile, :], pstrip[:, : ntile * P]
            )
            pv = ps_pv.tile([P, D], F32, tag="pv")
            for ti in range(ntile):
                nc.tensor.matmul(
                    pv[:],
                    pts[:, ti, :],
                    xh[:, ti, :],
                    start=(ti == 0),
                    stop=(ti == ntile - 1),
                )
            ob = o_pool.tile([P, D], F32, tag="ob")
            nc.scalar.activation(
                ob[:],
                pv[:],
                mybir.ActivationFunctionType.Copy,
                bias=0.0,
                scale=rcp[:],
            )
            nc.sync.dma_start(o_d[qi2 * P : (qi2 + 1) * P, :], ob[:])


_COMPILED = None


def _get_compiled():
    global _COMPILED
    if _COMPILED is None:
        nc = bacc.Bacc("TRN2", target_bir_lowering=False, debug=False)
        _emit(nc)
        nc.compile()
        _COMPILED = nc
    return _COMPILED


def kernel(x: np.ndarray) -> np.ndarray:
    assert x.shape == (B, S, D), x.shape
    nc = _get_compiled()
    in_maps = [
        {"x": np.ascontiguousarray(x[b], dtype=np.float32)} for b in range(B)
    ]
    res = run_bass_kernel_spmd(nc, in_maps, core_ids=list(range(B)))
    return np.stack([res.results[b]["out"] for b in range(B)], axis=0)


# revision 16
# speedup vs baseline: 1.2920x; 1.2920x over previous
"""Causal self-attention (Q=K=V=x, unscaled) on 8 trn2 NeuronCores.

x: [8, 2048, 512] f32. Data-parallel over batch: core b computes batch
element b entirely on-chip. fp16 matmul operands, f32 PSUM accumulation
and f32 softmax arithmetic:

  setup   x -> xh fp16 directly via casting SWDGE DMAs;
          xth = x.T fp16 via PE transposes
  scores  S = x @ x.T causal lower triangle, accumulated in 1024-wide
          (two-bank) PSUM stripes
  softmax causal mask added in-PSUM (DVE), per-stripe partial row-maxes
          (DVE), exp reads PSUM directly (ACT) -> fp16 P strip with
          fused per-stripe row-sums
  out     P tiles PE-transposed (fp16, 1 cyc/row) into a whole-strip
          two-bank PSUM tile ahead of the next block's score matmuls,
          so the PSUM->SBUF copy of P^T overlaps them; P @ x
          accumulates in PSUM; 1/rowsum fused into the output copy
"""

import numpy as np

import concourse.bass as bass
import concourse.mybir as mybir
import concourse.tile as tile
from concourse import bacc
from concourse.bass_utils import run_bass_kernel_spmd
from concourse.masks import make_causal_mask, make_identity

B, S, D = 8, 2048, 512
P = 128
NQ = S // P  # 16 q-blocks of 128 rows
ND = D // P  # 4 contraction chunks of 128
CW = 512  # matmul moving-dim chunk (one PSUM bank of f32)
SW = 1024  # softmax stripe width (two PSUM banks)
F32 = mybir.dt.float32
F16 = mybir.dt.float16
MASK_VAL = -1e30


def _emit(nc: bass.Bass, reps: int = 1):
    x_d = nc.dram_tensor("x", [S, D], F32, kind="ExternalInput").ap()
    o_d = nc.dram_tensor("out", [S, D], F32, kind="ExternalOutput").ap()

    with tile.TileContext(nc) as tc:
        with (
            tc.tile_pool(name="const", bufs=1) as cpool,
            tc.tile_pool(name="xsb", bufs=1) as x_pool,
            tc.tile_pool(name="pstrip", bufs=2) as sc_pool,
            tc.tile_pool(name="pts", bufs=4) as pt_pool,
            tc.tile_pool(name="ob", bufs=2) as o_pool,
            tc.tile_pool(name="stat", bufs=2) as st_pool,
            tc.tile_pool(name="ps_sc", bufs=2, space="PSUM") as ps_sc,
            tc.tile_pool(name="ps_tp", bufs=2, space="PSUM") as ps_tp,
            tc.tile_pool(name="ps_pv", bufs=2, space="PSUM") as ps_pv,
        ):
            ident = cpool.tile([P, P], F16, tag="ident")
            make_identity(nc, ident[:])
            cmask = cpool.tile([P, P], F32, tag="cmask")
            make_causal_mask(nc, cmask[:], mask_val=MASK_VAL)

            if reps > 1:
                # benchmarking only: repeat the whole body in a HW loop
                import contextlib  # noqa: F401

                loop_cm = tc.For_i(
                    0, reps, 1, hint_engines=(mybir.EngineType.PE,)
                )
            else:
                import contextlib

                loop_cm = contextlib.nullcontext()
            with loop_cm:
                _emit_body(nc, tc, x_d, o_d, ident, cmask, x_pool, sc_pool,
                           pt_pool, o_pool, st_pool, ps_sc, ps_tp, ps_pv)


def _emit_body(nc, tc, x_d, o_d, ident, cmask, x_pool, sc_pool, pt_pool,
               o_pool, st_pool, ps_sc, ps_tp, ps_pv):
    # xh: x in fp16 [t=128, ti, d]; xth: x.T in fp16 [d=128, dk, t]
    xh = x_pool.tile([P, NQ, D], F16, tag="xh")
    xth = x_pool.tile([P, ND, S], F16, tag="xth")
    x_blk = x_d.rearrange("(n p) d -> p n d", p=P)

    def emit_setup_group(tg):
        # rows tg*512..(tg+1)*512: casting DMA straight into fp16 SBUF,
        # then PE-transpose 128x128 blocks into xth.
        nc.gpsimd.dma_start(
            xh[:, tg * 4 : (tg + 1) * 4, :],
            x_blk[:, tg * 4 : (tg + 1) * 4, :],
        )
        for dk in range(ND):
            tp = ps_tp.tile([P, CW], F16, tag="tp", name=f"xtp{tg}_{dk}")
            for j in range(4):
                ti = tg * 4 + j
                nc.tensor.transpose(
                    tp[:, j * P : (j + 1) * P],
                    xh[:, ti, dk * P : (dk + 1) * P],
                    ident[:],
                )
            nc.vector.tensor_copy(
                xth[:, dk, tg * CW : (tg + 1) * CW], tp[:]
            )

    # Software pipeline, stage s:
    #   PE order: transposes of P(s-1) -> scores(s) -> PV(s-1)
    # so the DVE copy of P^T(s-1) overlaps scores(s) matmuls, and the
    # DVE/ACT softmax of block s overlaps PV(s-1) + scores(s+1).
    state = [None] * NQ
    for step in range(NQ + 1):
        pv_info = None
        if step >= 1:
            qi2 = step - 1
            pstrip, rcp = state[qi2]
            state[qi2] = None
            ntile = qi2 + 1
            # transpose P tiles of block s-1 in groups of 4; the DVE
            # copies into SBUF overlap scores(s) matmuls below
            pts_groups = []
            for g0 in range(0, ntile, 4):
                gn = min(4, ntile - g0)
                tp = ps_tp.tile([P, CW], F16, tag="tp", name=f"ptp{qi2}_{g0}")
                for j in range(gn):
                    ti = g0 + j
                    nc.tensor.transpose(
                        tp[:, j * P : (j + 1) * P],
                        pstrip[:, ti * P : (ti + 1) * P],
                        ident[:],
                    )
                pts = pt_pool.tile([P, CW], F16, tag="pts")
                nc.vector.tensor_copy(pts[:, : gn * P], tp[:, : gn * P])
                pts_groups.append((g0, gn, pts))
            pv_info = (qi2, ntile, pts_groups, rcp)

        if step < NQ:
            qi = step
            if qi % 4 == 0:
                emit_setup_group(qi // 4)
            width = (qi + 1) * P
            pstrip = sc_pool.tile([P, S], F16, tag="pstrip")
            nstripe = (width + SW - 1) // SW
            pmax = st_pool.tile([P, 2], F32, tag="pmax")
            stripes = []
            for c in range(nstripe):
                lo = c * SW
                sw = min(SW, width - lo)
                ps = ps_sc.tile([P, SW], F32, tag="ps", name=f"ps{qi}_{c}")
                for h in range(0, sw, CW):
                    cw = min(CW, sw - h)
                    for dk in range(ND):
                        nc.tensor.matmul(
                            ps[:, h : h + cw],
                            xth[:, dk, qi * P : (qi + 1) * P],
                            xth[:, dk, lo + h : lo + h + cw],
                            start=(dk == 0),
                            stop=(dk == ND - 1),
                        )
                if lo + sw > qi * P:
                    # stripe holds the diagonal 128x128 tile: apply the
                    # causal mask in place in PSUM
                    doff = qi * P - lo
                    nc.vector.tensor_add(
                        ps[:, doff : doff + P],
                        ps[:, doff : doff + P],
                        cmask[:],
                    )
                nc.vector.reduce_max(
                    pmax[:, c : c + 1],
                    ps[:, :sw],
                    axis=mybir.AxisListType.X,
                )
                stripes.append((ps, lo, sw))
            nmax = st_pool.tile([P, 1], F32, tag="nmax")
            nc.vector.reduce_max(
                nmax[:],
                pmax[:, :nstripe],
                axis=mybir.AxisListType.X,
                negate=True,
            )
            psums = st_pool.tile([P, 2], F32, tag="psums")
            for c, (ps, lo, sw) in enumerate(stripes):
                nc.scalar.activation(
                    pstrip[:, lo : lo + sw],
                    ps[:, :sw],
                    mybir.ActivationFunctionType.Exp,
                    bias=nmax[:],
                    scale=1.0,
                    accum_out=psums[:, c : c + 1],
                )
            rsum = st_pool.tile([P, 1], F32, tag="rsum")
            nc.vector.reduce_sum(
                rsum[:], psums[:, :nstripe], axis=mybir.AxisListType.X
            )
            rcp = st_pool.tile([P, 1], F32, tag="rcp")
            nc.vector.reciprocal(rcp[:], rsum[:])
            state[qi] = (pstrip, rcp)

        if pv_info is not None:
            qi2, ntile, pts_groups, rcp = pv_info
            pv = ps_pv.tile([P, D], F32, tag="pv")
            for g0, gn, pts in pts_groups:
                for j in range(gn):
                    ti = g0 + j
                    nc.tensor.matmul(
                        pv[:],
                        pts[:, j * P : (j + 1) * P],
                        xh[:, ti, :],
                        start=(ti == 0),
                        stop=(ti == ntile - 1),
                    )
            ob = o_pool.tile([P, D], F32, tag="ob")
            nc.scalar.activation(
                ob[:],
                pv[:],
                mybir.ActivationFunctionType.Copy,
                bias=0.0,
                scale=rcp[:],
            )
            nc.sync.dma_start(o_d[qi2 * P : (qi2 + 1) * P, :], ob[:])


_COMPILED = None


def _get_compiled():
    global _COMPILED
    if _COMPILED is None:
        nc = bacc.Bacc("TRN2", target_bir_lowering=False, debug=False)
        _emit(nc)
        nc.compile()
        _COMPILED = nc
    return _COMPILED


def kernel(x: np.ndarray) -> np.ndarray:
    assert x.shape == (B, S, D), x.shape
    nc = _get_compiled()
    in_maps = [
        {"x": np.ascontiguousarray(x[b], dtype=np.float32)} for b in range(B)
    ]
    res = run_bass_kernel_spmd(nc, in_maps, core_ids=list(range(B)))
    return np.stack([res.results[b]["out"] for b in range(B)], axis=0)


# revision 18
# speedup vs baseline: 1.4853x; 1.1496x over previous
"""Causal self-attention (Q=K=V=x, unscaled) on 8 trn2 NeuronCores.

x: [8, 2048, 512] f32. Data-parallel over batch: core b computes batch
element b entirely on-chip. fp16 matmul operands, f32 PSUM accumulation
and f32 softmax arithmetic:

  setup   x -> xh fp16 directly via casting SWDGE DMAs (prefetched two
          pipeline stages ahead); xth = x.T fp16 via PE transposes
  scores  S = x @ x.T causal lower triangle, accumulated in 1024-wide
          (two-bank) PSUM stripes
  softmax causal mask added in-PSUM (DVE), per-stripe partial row-maxes
          (DVE), exp reads PSUM directly (ACT) -> fp16 P strip with
          fused per-stripe row-sums
  out     P tiles PE-transposed (fp16, 1 cyc/row) in groups of 4; the
          transposes are emitted after the next block's first score
          stripe so the softmax tail and the P^T PSUM->SBUF copies hide
          under score matmuls; P @ x accumulates in PSUM; 1/rowsum is
          fused into the output copy (ACT scale)
"""

import numpy as np

import concourse.bass as bass
import concourse.mybir as mybir
import concourse.tile as tile
from concourse import bacc
from concourse.bass_utils import run_bass_kernel_spmd
from concourse.masks import make_causal_mask, make_identity

B, S, D = 8, 2048, 512
P = 128
NQ = S // P  # 16 q-blocks of 128 rows
ND = D // P  # 4 contraction chunks of 128
CW = 512  # matmul moving-dim chunk (one PSUM bank of f32)
SW = 1024  # softmax stripe width (two PSUM banks)
F32 = mybir.dt.float32
F16 = mybir.dt.float16
MASK_VAL = -1e30


def _emit(nc: bass.Bass, reps: int = 1):
    x_d = nc.dram_tensor("x", [S, D], F32, kind="ExternalInput").ap()
    o_d = nc.dram_tensor("out", [S, D], F32, kind="ExternalOutput").ap()

    with tile.TileContext(nc) as tc:
        with (
            tc.tile_pool(name="const", bufs=1) as cpool,
            tc.tile_pool(name="xsb", bufs=1) as x_pool,
            tc.tile_pool(name="pstrip", bufs=2) as sc_pool,
            tc.tile_pool(name="pts", bufs=4) as pt_pool,
            tc.tile_pool(name="ob", bufs=2) as o_pool,
            tc.tile_pool(name="stat", bufs=2) as st_pool,
            tc.tile_pool(name="ps_sc", bufs=2, space="PSUM") as ps_sc,
            tc.tile_pool(name="ps_tp", bufs=2, space="PSUM") as ps_tp,
            tc.tile_pool(name="ps_pv", bufs=2, space="PSUM") as ps_pv,
        ):
            ident = cpool.tile([P, P], F16, tag="ident")
            make_identity(nc, ident[:])
            cmask = cpool.tile([P, P], F32, tag="cmask")
            make_causal_mask(nc, cmask[:], mask_val=MASK_VAL)

            if reps > 1:
                # benchmarking only: repeat the whole body in a HW loop
                import contextlib  # noqa: F401

                loop_cm = tc.For_i(
                    0, reps, 1, hint_engines=(mybir.EngineType.PE,)
                )
            else:
                import contextlib

                loop_cm = contextlib.nullcontext()
            with loop_cm:
                _emit_body(nc, tc, x_d, o_d, ident, cmask, x_pool, sc_pool,
                           pt_pool, o_pool, st_pool, ps_sc, ps_tp, ps_pv)


def _emit_body(nc, tc, x_d, o_d, ident, cmask, x_pool, sc_pool, pt_pool,
               o_pool, st_pool, ps_sc, ps_tp, ps_pv):
    # xh: x in fp16 [t=128, ti, d]; xth: x.T in fp16 [d=128, dk, t]
    xh = x_pool.tile([P, NQ, D], F16, tag="xh")
    xth = x_pool.tile([P, ND, S], F16, tag="xth")
    x_blk = x_d.rearrange("(n p) d -> p n d", p=P)

    def emit_cast_dma(tg):
        # casting DMAs straight into fp16 SBUF, one per 128-row block
        for j in range(4):
            ti = tg * 4 + j
            nc.gpsimd.dma_start(xh[:, ti, :], x_blk[:, ti, :])

    def emit_setup_transposes(tg):
        for dk in range(ND):
            tp = ps_tp.tile([P, CW], F16, tag="tp", name=f"xtp{tg}_{dk}")
            for j in range(4):
                ti = tg * 4 + j
                nc.tensor.transpose(
                    tp[:, j * P : (j + 1) * P],
                    xh[:, ti, dk * P : (dk + 1) * P],
                    ident[:],
                )
            nc.vector.tensor_copy(
                xth[:, dk, tg * CW : (tg + 1) * CW], tp[:]
            )

    def emit_score_stripe(qi, c, width, pstrip, pmax, stripes):
        lo = c * SW
        sw = min(SW, width - lo)
        ps = ps_sc.tile([P, SW], F32, tag="ps", name=f"ps{qi}_{c}")
        for h in range(0, sw, CW):
            cw = min(CW, sw - h)
            for dk in range(ND):
                nc.tensor.matmul(
                    ps[:, h : h + cw],
                    xth[:, dk, qi * P : (qi + 1) * P],
                    xth[:, dk, lo + h : lo + h + cw],
                    start=(dk == 0),
                    stop=(dk == ND - 1),
                )
        if lo + sw > qi * P:
            # stripe holds the diagonal 128x128 tile: apply the causal
            # mask in place in PSUM
            doff = qi * P - lo
            nc.vector.tensor_add(
                ps[:, doff : doff + P], ps[:, doff : doff + P], cmask[:]
            )
        nc.vector.reduce_max(
            pmax[:, c : c + 1], ps[:, :sw], axis=mybir.AxisListType.X
        )
        stripes.append((ps, lo, sw))

    def emit_softmax_tail(qi, pstrip, pmax, stripes):
        nstripe = len(stripes)
        nmax = st_pool.tile([P, 1], F32, tag="nmax")
        nc.vector.reduce_max(
            nmax[:], pmax[:, :nstripe], axis=mybir.AxisListType.X,
            negate=True,
        )
        psums = st_pool.tile([P, 2], F32, tag="psums")
        for c, (ps, lo, sw) in enumerate(stripes):
            nc.scalar.activation(
                pstrip[:, lo : lo + sw],
                ps[:, :sw],
                mybir.ActivationFunctionType.Exp,
                bias=nmax[:],
                scale=1.0,
                accum_out=psums[:, c : c + 1],
            )
        rsum = st_pool.tile([P, 1], F32, tag="rsum")
        nc.vector.reduce_sum(
            rsum[:], psums[:, :nstripe], axis=mybir.AxisListType.X
        )
        rcp = st_pool.tile([P, 1], F32, tag="rcp")
        nc.vector.reciprocal(rcp[:], rsum[:])
        return rcp

    def emit_p_transposes(qi2, pstrip):
        ntile = qi2 + 1
        pts_groups = []
        for g0 in range(0, ntile, 4):
            gn = min(4, ntile - g0)
            tp = ps_tp.tile([P, CW], F16, tag="tp", name=f"ptp{qi2}_{g0}")
            for j in range(gn):
                ti = g0 + j
                nc.tensor.transpose(
                    tp[:, j * P : (j + 1) * P],
                    pstrip[:, ti * P : (ti + 1) * P],
                    ident[:],
                )
            pts = pt_pool.tile([P, CW], F16, tag="pts")
            nc.vector.tensor_copy(pts[:, : gn * P], tp[:, : gn * P])
            pts_groups.append((g0, gn, pts))
        return pts_groups

    def emit_pv(qi2, pts_groups, rcp):
        ntile = qi2 + 1
        pv = ps_pv.tile([P, D], F32, tag="pv")
        for g0, gn, pts in pts_groups:
            for j in range(gn):
                ti = g0 + j
                nc.tensor.matmul(
                    pv[:],
                    pts[:, j * P : (j + 1) * P],
                    xh[:, ti, :],
                    start=(ti == 0),
                    stop=(ti == ntile - 1),
                )
        ob = o_pool.tile([P, D], F32, tag="ob")
        nc.scalar.activation(
            ob[:],
            pv[:],
            mybir.ActivationFunctionType.Copy,
            bias=0.0,
            scale=rcp[:],
        )
        nc.sync.dma_start(o_d[qi2 * P : (qi2 + 1) * P, :], ob[:])

    emit_cast_dma(0)
    emit_cast_dma(1)

    state = [None] * NQ
    for step in range(NQ + 1):
        # prefetch the casting DMAs two stages ahead of first use
        if step in (6, 10):
            emit_cast_dma((step + 2) // 4)

        if step < NQ:
            qi = step
            if qi % 4 == 0:
                emit_setup_transposes(qi // 4)
            width = (qi + 1) * P
            pstrip = sc_pool.tile([P, S], F16, tag="pstrip")
            pmax = st_pool.tile([P, 2], F32, tag="pmax")
            stripes = []
            nstripe = (width + SW - 1) // SW
            # first stripe of scores(s) runs on PE before the transposes
            # of P(s-1), giving softmax(s-1) time to finish
            emit_score_stripe(qi, 0, width, pstrip, pmax, stripes)
            if step >= 1:
                qi2 = step - 1
                prev_pstrip, prev_rcp = state[qi2]
                state[qi2] = None
                pts_groups = emit_p_transposes(qi2, prev_pstrip)
            for c in range(1, nstripe):
                emit_score_stripe(qi, c, width, pstrip, pmax, stripes)
            rcp = emit_softmax_tail(qi, pstrip, pmax, stripes)
            state[qi] = (pstrip, rcp)
            if step >= 1:
                emit_pv(qi2, pts_groups, prev_rcp)
        else:
            qi2 = step - 1
            prev_pstrip, prev_rcp = state[qi2]
            state[qi2] = None
            pts_groups = emit_p_transposes(qi2, prev_pstrip)
            emit_pv(qi2, pts_groups, prev_rcp)


_COMPILED = None


def _get_compiled():
    global _COMPILED
    if _COMPILED is None:
        nc = bacc.Bacc("TRN2", target_bir_lowering=False, debug=False)
        _emit(nc)
        nc.compile()
        _COMPILED = nc
    return _COMPILED


def kernel(x: np.ndarray) -> np.ndarray:
    assert x.shape == (B, S, D), x.shape
    nc = _get_compiled()
    in_maps = [
        {"x": np.ascontiguousarray(x[b], dtype=np.float32)} for b in range(B)
    ]
    res = run_bass_kernel_spmd(nc, in_maps, core_ids=list(range(B)))
    return np.stack([res.results[b]["out"] for b in range(B)], axis=0)
